# revision 7
# baseline (speedup 1.0000x reference)
"""AvgDistanceConv (GNN message passing) on 8 Trainium2 NeuronCores.

out[:, 0] = pos = h[:, 0]
out[:, 1] = segment_mean over incoming edges of |pos[src] - pos[dst]|

Strategy
--------
Shard by destination range: core c owns nodes [c*12500, (c+1)*12500) and
processes exactly the edges pointing into them (no collectives).

The per-edge gather of pos[src] runs as GPSIMD ap_gather ucode (SBUF->SBUF,
measured 27.2 ns/idx per Q7 core, 8 cores in parallel) instead of
per-element indirect DMA (994 ns SWDGE overhead per 128 elements -> 8.9 ms).

Layout: the core's 12500 dst nodes are placed degree-sorted into 98
iterations x 128 partitions; partition 16g+r belongs to GPSIMD core
(group) g. Edges are split into 8 passes by src chunk of 12500 so each
pass's pos chunk fits the ap_gather table (<=32768 elems, int16 idx).
Per (pass, iteration) each group gathers the unpadded concatenation of its
16 channels' edge-source lists; ap_gather replicates the stream across the
group's 16 channels, so channel r picks out its own segment with a
host-staged 0/1 bf16 mask (index-derived): per iteration the DVE computes
|(v - pos_dst) * mask| and abs-sum-reduces to one scalar per channel.
Sums accumulate in f32; a final reciprocal-multiply divides by in-degree.
Host work is index-only plus O(N) float permutations of pos.
"""
import sys
sys.path.insert(0, '/opt/trn_rl_repo')
import numpy as np
import ml_dtypes
import concourse.bass as bass
import concourse.bacc as bacc
import concourse.mybir as mybir
from concourse.bass_utils import run_bass_kernel_spmd
from concourse.tile import TileContext

P = 128
NC = 8
N_NODES = 100000
NPC = N_NODES // NC          # 12500 dst nodes per core
ITERS = (NPC + P - 1) // P   # 98 iterations (12544 slots, 44 dummies)
PASSES = 8
CPC = N_NODES // PASSES      # 12500-wide src chunks per pass
IDX_CAP = 4096               # max num_idxs per ap_gather instruction
BF = ml_dtypes.bfloat16


def _split_sync_waits(nc, max_waits=1):
    """This walrus build rejects more than one sync wait per instruction.
    Hoist extras into standalone same-engine EventSemaphore waits placed
    immediately before the owning instruction (same-engine program order
    preserves the synchronization semantics)."""
    for f in nc.m.functions:
        for blk in f.blocks:
            insts = list(blk.instructions)
            new = []
            dirty = False
            for inst in insts:
                si = inst.sync_info
                if si is not None and len(si.on_wait) > max_waits:
                    waits = list(si.on_wait)
                    for j, w in enumerate(waits[:-max_waits]):
                        wi = mybir.InstEventSemaphore(
                            name=f"{inst.name}_hw{j}", ins=[], outs=[])
                        wi.engine = inst.engine
                        wi.sync_info = mybir.SyncInfo(on_wait=[w], on_update=[])
                        new.append(wi)
                    inst.sync_info = mybir.SyncInfo(
                        on_wait=waits[-max_waits:], on_update=list(si.on_update))
                    dirty = True
                new.append(inst)
            if dirty:
                blk.instructions = new


def _host_prep(h, src, dst):
    N = N_NODES
    pos = np.ascontiguousarray(h[:, 0], dtype=np.float32)
    src32 = src.astype(np.int32)
    dst32 = dst.astype(np.int32)

    deg = np.bincount(dst32, minlength=N)

    deg_c = deg.reshape(NC, NPC)
    rank = np.argsort(-deg_c, axis=1, kind='stable')          # [NC, NPC]
    node_ids = rank + (np.arange(NC)[:, None] * NPC)
    pad = ITERS * P - NPC
    # pad with sentinel node id N (zero-degree dummy; posx/degx extended)
    nodes_rank = np.concatenate(
        [node_ids, np.full((NC, pad), N, np.int64)], axis=1
    ).reshape(NC, ITERS, P)

    posx = np.append(pos, np.float32(0.0))
    degx = np.append(deg, 0)

    # per-(node, pass) in-degree, for balancing groups within an iteration
    e_pass0 = (src32 // CPC).astype(np.int64)
    degp = np.bincount(dst32.astype(np.int64) * PASSES + e_pass0,
                       minlength=(N + 1) * PASSES).reshape(N + 1, PASSES)

    # greedy LPT: assign each iteration's 128 nodes to 8 groups x 16 slots,
    # minimizing the max per-(group, pass) load (the shared window width L)
    nd = degp[nodes_rank]                                     # [NC, IT, 128, PASSES]
    ci, ii = np.ogrid[:NC, :ITERS]

    def greedy(ordj):
        """One LPT pass; returns (group choice per ordered node, score)."""
        loads = np.zeros((NC, ITERS, 8, PASSES), np.int64)
        sizes = np.zeros((NC, ITERS, 8), np.int64)
        gsel = np.empty((NC, ITERS, P), np.int8)
        for j in range(P):
            jj = ordj[:, :, j]
            d_j = nd[ci, ii, jj, :]                           # [NC, IT, PASSES]
            cand = loads + d_j[:, :, None, :]
            score = (cand.max(axis=3) * 4096 + cand.sum(axis=3)
                     + (sizes >= 16) * (1 << 50))
            g = score.argmin(axis=2)                          # [NC, IT]
            gsel[:, :, j] = g
            loads[ci, ii, g] += d_j
            sizes[ci, ii, g] += 1
        return gsel, loads.max(axis=2).sum(axis=2)            # [NC, IT]

    # multi-restart: different placement orders, keep best per (core, iter)
    rng = np.random.default_rng(12345)
    base = np.argsort(-nd.max(axis=3), axis=2, kind='stable')
    orders = [base, np.argsort(-nd.sum(axis=3), axis=2, kind='stable')]
    for _ in range(10):
        perm = rng.permuted(np.broadcast_to(
            np.arange(P), (NC, ITERS, P)).copy(), axis=2)
        orders.append(perm)
    best_score = None
    best_g = None
    best_ord = None
    for ordj in orders:
        gsel, sc = greedy(ordj)
        if best_score is None:
            best_score, best_g, best_ord = sc, gsel, ordj.copy()
        else:
            upd = sc < best_score
            best_score = np.where(upd, sc, best_score)
            best_g[upd] = gsel[upd]
            best_ord[upd] = ordj[upd]

    # rebuild placement from the chosen restart per (core, iter)
    nodes_gic = np.empty((NC, ITERS, P), np.int64)
    sizes = np.zeros((NC, ITERS, 8), np.int64)
    for j in range(P):
        jj = best_ord[:, :, j]
        g = best_g[:, :, j].astype(np.int64)
        slot = sizes[ci, ii, g]
        nodes_gic[ci, ii, 16 * g + slot] = nodes_rank[ci, ii, jj]
        sizes[ci, ii, g] += 1

    # swap-refinement: for each (core, iter), repeatedly try swapping the
    # heaviest node (in the binding pass) of the most-loaded group with the
    # lightest node of the least-loaded group; accept if sum_p max_g improves
    ndg = degp[nodes_gic]                                     # [NC, IT, 128, PASSES]
    r16 = np.arange(16)
    for _ in range(48):
        loads = ndg.reshape(NC, ITERS, 8, 16, PASSES).sum(axis=3)
        score = loads.max(axis=2).sum(axis=2)                 # [NC, IT]
        p_star = loads.max(axis=2).argmax(axis=2)             # [NC, IT]
        lp = np.take_along_axis(
            loads, p_star[:, :, None, None], axis=3)[..., 0]  # [NC, IT, 8]
        g_hi = lp.argmax(axis=2)
        g_lo = lp.argmin(axis=2)
        dsp = np.take_along_axis(
            ndg, p_star[:, :, None, None], axis=3)[..., 0]    # [NC, IT, 128]
        hi_ch = 16 * g_hi[:, :, None] + r16
        lo_ch = 16 * g_lo[:, :, None] + r16
        a_ch = np.take_along_axis(
            np.take_along_axis(dsp, hi_ch, axis=2).argmax(axis=2)[:, :, None]
            + 16 * g_hi[:, :, None], np.zeros_like(g_hi)[:, :, None],
            axis=2)[..., 0]
        b_ch = np.take_along_axis(
            np.take_along_axis(dsp, lo_ch, axis=2).argmin(axis=2)[:, :, None]
            + 16 * g_lo[:, :, None], np.zeros_like(g_lo)[:, :, None],
            axis=2)[..., 0]
        da = np.take_along_axis(ndg, a_ch[:, :, None, None], axis=2)[:, :, 0]
        db = np.take_along_axis(ndg, b_ch[:, :, None, None], axis=2)[:, :, 0]
        delta = db - da                                       # [NC, IT, PASSES]
        new_loads = loads.copy()
        ciX, iiX = np.ogrid[:NC, :ITERS]
        new_loads[ciX, iiX, g_hi] += delta
        new_loads[ciX, iiX, g_lo] -= delta
        new_score = new_loads.max(axis=2).sum(axis=2)
        acc = new_score < score                               # [NC, IT]
        if not acc.any():
            break
        na = np.take_along_axis(nodes_gic, a_ch[:, :, None], axis=2)[..., 0]
        nb = np.take_along_axis(nodes_gic, b_ch[:, :, None], axis=2)[..., 0]
        np.put_along_axis(nodes_gic, a_ch[:, :, None],
                          np.where(acc, nb, na)[:, :, None], axis=2)
        np.put_along_axis(nodes_gic, b_ch[:, :, None],
                          np.where(acc, na, nb)[:, :, None], axis=2)
        accx = acc[:, :, None]
        np.put_along_axis(ndg, a_ch[:, :, None, None],
                          np.where(accx, db, da)[:, :, None], axis=2)
        np.put_along_axis(ndg, b_ch[:, :, None, None],
                          np.where(accx, da, db)[:, :, None], axis=2)

    # inverse: real node -> (iteration, channel)
    it_of = np.empty(N + 1, np.int32)
    ch_of = np.empty(N + 1, np.int32)
    for c in range(NC):
        flat = nodes_gic[c].reshape(-1)
        it_of[flat] = np.arange(ITERS * P) // P
        ch_of[flat] = np.arange(ITERS * P) % P

    W = posx[nodes_gic].transpose(0, 2, 1).copy()             # [NC, 128, 98]
    cntf = degx[nodes_gic].transpose(0, 2, 1).astype(np.float32)

    # per-edge placement
    e_core = dst32 // NPC
    e_it = it_of[dst32]
    e_ch = ch_of[dst32]
    e_grp = e_ch // 16
    e_r = e_ch % 16
    e_pass = src32 // CPC
    e_sidx = (src32 - e_pass * CPC).astype(np.int16)

    # group-stream length per (core, pass, group, iter) then shared width
    key = (((e_core.astype(np.int64) * PASSES + e_pass) * 8 + e_grp)
           * ITERS + e_it)
    glen = np.bincount(key, minlength=NC * PASSES * 8 * ITERS)
    glen = glen.reshape(NC, PASSES, 8, ITERS)
    # shared L per (pass, iter): max over cores and groups (no per-window
    # rounding -- masks/sidx are absolute-slot addressed; only each chunk's
    # total is padded to a multiple of 16 for the wrapped idx layout)
    L_pi = np.maximum(glen.max(axis=(0, 2)), 1).astype(np.int64)

    # chunking: pack iterations so the padded sum <= IDX_CAP
    chunks = []                                               # per pass: (it0, it1)
    for p in range(PASSES):
        ch_list = []
        it0 = 0
        while it0 < ITERS:
            tot = 0
            it1 = it0
            while it1 < ITERS and ((tot + int(L_pi[p, it1]) + 15) // 16 * 16
                                   <= IDX_CAP):
                tot += int(L_pi[p, it1])
                it1 += 1
            ch_list.append((it0, it1))
            it0 = it1
        chunks.append(ch_list)

    # column offsets per (pass, iter); chunks padded to x16
    colof = np.zeros((PASSES, ITERS), np.int64)
    chunk_cols = {}                                           # (p, it0) -> padded cols
    off = 0
    for p in range(PASSES):
        for (it0, it1) in chunks[p]:
            c0 = off
            for it in range(it0, it1):
                colof[p, it] = off
                off += int(L_pi[p, it])
            off = (off + 15) // 16 * 16
            chunk_cols[(p, it0)] = off - c0
    total_cols = off                                          # slots per group

    # edge slot position: order edges by (core, pass, grp, it, r) and number
    order = np.lexsort((e_r, e_it, e_grp, e_pass, e_core))
    ks = key[order]
    run_start = np.r_[True, ks[1:] != ks[:-1]]
    pos_in_grp = np.arange(len(order)) - np.maximum.accumulate(
        np.where(run_start, np.arange(len(order)), 0))
    # slot of each (sorted) edge within its (pass, it) stream window
    oc = e_core[order]
    op_ = e_pass[order]
    og = e_grp[order]
    oi = e_it[order]
    orr = e_r[order]
    slot = colof[op_, oi] + pos_in_grp                        # [E] global col

    # sidx [NC, 128, total_cols/16] int16, wrapped per group;
    # mask [NC, 128, total_cols] bf16
    sidx = np.zeros((NC, P, total_cols // 16), np.int16)
    mask = np.zeros((NC, P, total_cols), BF)
    # wrapped position: stream slot t -> (partition 16g + t%16, col t//16)
    sidx[oc, 16 * og + slot % 16, slot // 16] = e_sidx[order]
    mask[oc, 16 * og + orr, slot] = 1.0

    # pass tables [NC, PASSES, 128, CPC] f32 (pos chunk replicated; pad slots
    # are masked so table[0] garbage is harmless)
    tbl = np.empty((NC, PASSES, P, CPC), np.float32)
    for p in range(PASSES):
        tbl[:, p, :, :] = pos[p * CPC:(p + 1) * CPC][None, None, :]

    in_maps = []
    for c in range(NC):
        in_maps.append({
            "tbl": tbl[c].reshape(PASSES * P, CPC),
            "sidx": sidx[c],
            "mask": mask[c],
            "wtab": W[c],
            "cntf": cntf[c],
        })
    meta = dict(chunks=chunks, L_pi=L_pi, colof=colof, chunk_cols=chunk_cols,
                total_cols=int(total_cols), nodes_gic=nodes_gic)
    return in_maps, meta


def _build_program(meta):
    chunks, L_pi, total_cols = meta["chunks"], meta["L_pi"], meta["total_cols"]
    chunk_cols = meta["chunk_cols"]
    nc = bacc.Bacc()
    tbl = nc.declare_dram_parameter("tbl", [PASSES * P, CPC],
                                    mybir.dt.float32, isOutput=False)
    sidx = nc.declare_dram_parameter("sidx", [P, total_cols // 16],
                                     mybir.dt.int16, isOutput=False)
    mask = nc.declare_dram_parameter("mask", [P, total_cols],
                                     mybir.dt.bfloat16, isOutput=False)
    wtab = nc.declare_dram_parameter("wtab", [P, ITERS], mybir.dt.float32,
                                     isOutput=False)
    cntf = nc.declare_dram_parameter("cntf", [P, ITERS], mybir.dt.float32,
                                     isOutput=False)
    out = nc.declare_dram_parameter("out", [P, 2 * ITERS], mybir.dt.float32,
                                    isOutput=True)
    outv = out[:].rearrange("p (b a) -> p b a", b=2)

    with TileContext(nc) as tc:
        with (
            tc.tile_pool(name="persist", bufs=1) as pers,
            tc.tile_pool(name="tblp", bufs=2) as tblp,
            tc.tile_pool(name="idxp", bufs=4) as idxp,
            tc.tile_pool(name="maskp", bufs=3) as maskp,
            tc.tile_pool(name="vp", bufs=3) as vp,
            tc.tile_pool(name="tbp", bufs=2) as tbp,
            tc.tile_pool(name="sp", bufs=2) as sp,
        ):
            t_w = pers.tile([P, ITERS], mybir.dt.float32, tag="t_w")
            t_cnt = pers.tile([P, ITERS], mybir.dt.float32, tag="t_cnt")
            t_s = pers.tile([P, ITERS], mybir.dt.float32, tag="t_s")
            # pass-0 table first: it gates the first gather
            t_tbl0 = tblp.tile([P, CPC], mybir.dt.float32, tag="tbl")
            nc.sync.dma_start(out=t_tbl0[:], in_=tbl[0:P])
            nc.sync.dma_start(out=t_w[:], in_=wtab[:])
            nc.sync.dma_start(out=t_cnt[:], in_=cntf[:])
            nc.vector.memset(t_s[:], 0.0)

            off = 0
            for p in range(PASSES):
                if p == 0:
                    t_tbl = t_tbl0
                else:
                    t_tbl = tblp.tile([P, CPC], mybir.dt.float32, tag="tbl")
                    nc.sync.dma_start(out=t_tbl[:],
                                      in_=tbl[p * P:(p + 1) * P])
                s_cols = sp.tile([P, ITERS], mybir.dt.float32, tag="scols")
                nc.vector.memset(s_cols[:], 0.0)
                for (it0, it1) in chunks[p]:
                    Ls = [int(L_pi[p, it]) for it in range(it0, it1)]
                    cols = chunk_cols[(p, it0)]
                    si = idxp.tile([P, cols // 16], mybir.dt.int16, tag="si")
                    nc.sync.dma_start(out=si[:],
                                      in_=sidx[:, off // 16:(off + cols) // 16])
                    mk = maskp.tile([P, cols], mybir.dt.bfloat16, tag="mk")
                    nc.sync.dma_start(out=mk[:], in_=mask[:, off:off + cols])
                    v = vp.tile([P, cols], mybir.dt.float32, tag="v")
                    nc.gpsimd.ap_gather(out_ap=v[:], in_ap=t_tbl[:],
                                        idxs_ap=si[:], channels=P,
                                        num_elems=CPC, d=1, num_idxs=cols)
                    tb = tbp.tile([P, cols], mybir.dt.bfloat16, tag="tb")
                    co = 0
                    for k, it in enumerate(range(it0, it1)):
                        L = Ls[k]
                        nc.vector.tensor_scalar(
                            out=tb[:, co:co + L], in0=v[:, co:co + L],
                            scalar1=t_w[:, it:it + 1], scalar2=None,
                            op0=mybir.AluOpType.subtract)
                        nc.vector.tensor_tensor(
                            out=tb[:, co:co + L], in0=tb[:, co:co + L],
                            in1=mk[:, co:co + L], op=mybir.AluOpType.mult)
                        nc.vector.tensor_reduce(
                            out=s_cols[:, it:it + 1], in_=tb[:, co:co + L],
                            axis=mybir.AxisListType.X, op=mybir.AluOpType.add,
                            apply_absolute_value=True)
                        co += L
                    off += cols
                nc.vector.tensor_tensor(out=t_s[:], in0=t_s[:], in1=s_cols[:],
                                        op=mybir.AluOpType.add)

            nc.vector.tensor_scalar_max(out=t_cnt[:], in0=t_cnt[:],
                                        scalar1=1.0)
            nc.vector.reciprocal(out=t_cnt[:], in_=t_cnt[:])
            nc.vector.tensor_tensor(out=t_s[:], in0=t_s[:], in1=t_cnt[:],
                                    op=mybir.AluOpType.mult)
            nc.sync.dma_start(out=outv[:, 0], in_=t_w[:])
            nc.sync.dma_start(out=outv[:, 1], in_=t_s[:])

    nc.compile()
    _split_sync_waits(nc)
    return nc


def kernel(h, src, dst):
    h = np.asarray(h)
    src = np.asarray(src)
    dst = np.asarray(dst)
    in_maps, meta = _host_prep(h, src, dst)
    nc = _build_program(meta)
    res = run_bass_kernel_spmd(nc, in_maps, list(range(NC)))
    nodes_gic = meta["nodes_gic"]
    final = np.empty((N_NODES, 2), np.float32)
    for c in range(NC):
        r = res.results[c]["out"].reshape(P, 2, ITERS)
        flat_nodes = nodes_gic[c].reshape(-1)          # (it, ch) order
        vals = r.transpose(2, 0, 1).reshape(-1, 2)
        valid = flat_nodes < N_NODES                   # drop sentinel dummies
        final[flat_nodes[valid]] = vals[valid]
    return final


# revision 8
# speedup vs baseline: 1.0063x; 1.0063x over previous
"""AvgDistanceConv (GNN message passing) on 8 Trainium2 NeuronCores.

out[:, 0] = pos = h[:, 0]
out[:, 1] = segment_mean over incoming edges of |pos[src] - pos[dst]|

Strategy
--------
Shard by destination range: core c owns nodes [c*12500, (c+1)*12500) and
processes exactly the edges pointing into them (no collectives).

The per-edge gather of pos[src] runs as GPSIMD ap_gather ucode (SBUF->SBUF,
measured 27.2 ns/idx per Q7 core, 8 cores in parallel) instead of
per-element indirect DMA (994 ns SWDGE overhead per 128 elements -> 8.9 ms).

Layout: the core's 12500 dst nodes are placed degree-sorted into 98
iterations x 128 partitions; partition 16g+r belongs to GPSIMD core
(group) g. Edges are split into 8 passes by src chunk of 12500 so each
pass's pos chunk fits the ap_gather table (<=32768 elems, int16 idx).
Per (pass, iteration) each group gathers the unpadded concatenation of its
16 channels' edge-source lists; ap_gather replicates the stream across the
group's 16 channels, so channel r picks out its own segment with a
host-staged 0/1 bf16 mask (index-derived): per iteration the DVE computes
|(v - pos_dst) * mask| and abs-sum-reduces to one scalar per channel.
Sums accumulate in f32; a final reciprocal-multiply divides by in-degree.
Host work is index-only plus O(N) float permutations of pos.
"""
import sys
sys.path.insert(0, '/opt/trn_rl_repo')
import numpy as np
import ml_dtypes
import concourse.bass as bass
import concourse.bacc as bacc
import concourse.mybir as mybir
from concourse.bass_utils import run_bass_kernel_spmd
from concourse.tile import TileContext

P = 128
NC = 8
N_NODES = 100000
NPC = N_NODES // NC          # 12500 dst nodes per core
ITERS = (NPC + P - 1) // P   # 98 iterations (12544 slots, 44 dummies)
PASSES = 8
CPC = N_NODES // PASSES      # 12500-wide src chunks per pass
IDX_CAP = 4096               # max num_idxs per ap_gather instruction
BF = ml_dtypes.bfloat16


def _split_sync_waits(nc, max_waits=1):
    """This walrus build rejects more than one sync wait per instruction.
    Hoist extras into standalone same-engine EventSemaphore waits placed
    immediately before the owning instruction (same-engine program order
    preserves the synchronization semantics)."""
    for f in nc.m.functions:
        for blk in f.blocks:
            insts = list(blk.instructions)
            new = []
            dirty = False
            for inst in insts:
                si = inst.sync_info
                if si is not None and len(si.on_wait) > max_waits:
                    waits = list(si.on_wait)
                    for j, w in enumerate(waits[:-max_waits]):
                        wi = mybir.InstEventSemaphore(
                            name=f"{inst.name}_hw{j}", ins=[], outs=[])
                        wi.engine = inst.engine
                        wi.sync_info = mybir.SyncInfo(on_wait=[w], on_update=[])
                        new.append(wi)
                    inst.sync_info = mybir.SyncInfo(
                        on_wait=waits[-max_waits:], on_update=list(si.on_update))
                    dirty = True
                new.append(inst)
            if dirty:
                blk.instructions = new


def _host_prep(h, src, dst):
    N = N_NODES
    pos = np.ascontiguousarray(h[:, 0], dtype=np.float32)
    src32 = src.astype(np.int32)
    dst32 = dst.astype(np.int32)

    deg = np.bincount(dst32, minlength=N)

    deg_c = deg.reshape(NC, NPC)
    rank = np.argsort(-deg_c, axis=1, kind='stable')          # [NC, NPC]
    node_ids = rank + (np.arange(NC)[:, None] * NPC)
    pad = ITERS * P - NPC
    # pad with sentinel node id N (zero-degree dummy; posx/degx extended)
    nodes_rank = np.concatenate(
        [node_ids, np.full((NC, pad), N, np.int64)], axis=1
    ).reshape(NC, ITERS, P)

    posx = np.append(pos, np.float32(0.0))
    degx = np.append(deg, 0)

    # per-(node, pass) in-degree, for balancing groups within an iteration
    e_pass0 = (src32 // CPC).astype(np.int64)
    degp = np.bincount(dst32.astype(np.int64) * PASSES + e_pass0,
                       minlength=(N + 1) * PASSES).reshape(N + 1, PASSES)

    # greedy LPT: assign each iteration's 128 nodes to 8 groups x 16 slots,
    # minimizing the max per-(group, pass) load (the shared window width L)
    nd = degp[nodes_rank]                                     # [NC, IT, 128, PASSES]
    ci, ii = np.ogrid[:NC, :ITERS]

    def greedy(ordj):
        """One LPT pass; returns (group choice per ordered node, score)."""
        loads = np.zeros((NC, ITERS, 8, PASSES), np.int64)
        sizes = np.zeros((NC, ITERS, 8), np.int64)
        gsel = np.empty((NC, ITERS, P), np.int8)
        for j in range(P):
            jj = ordj[:, :, j]
            d_j = nd[ci, ii, jj, :]                           # [NC, IT, PASSES]
            cand = loads + d_j[:, :, None, :]
            score = (cand.max(axis=3) * 4096 + cand.sum(axis=3)
                     + (sizes >= 16) * (1 << 50))
            g = score.argmin(axis=2)                          # [NC, IT]
            gsel[:, :, j] = g
            loads[ci, ii, g] += d_j
            sizes[ci, ii, g] += 1
        return gsel, loads.max(axis=2).sum(axis=2)            # [NC, IT]

    # multi-restart: different placement orders, keep best per (core, iter)
    rng = np.random.default_rng(12345)
    base = np.argsort(-nd.max(axis=3), axis=2, kind='stable')
    orders = [base, np.argsort(-nd.sum(axis=3), axis=2, kind='stable')]
    for _ in range(10):
        perm = rng.permuted(np.broadcast_to(
            np.arange(P), (NC, ITERS, P)).copy(), axis=2)
        orders.append(perm)
    best_score = None
    best_g = None
    best_ord = None
    for ordj in orders:
        gsel, sc = greedy(ordj)
        if best_score is None:
            best_score, best_g, best_ord = sc, gsel, ordj.copy()
        else:
            upd = sc < best_score
            best_score = np.where(upd, sc, best_score)
            best_g[upd] = gsel[upd]
            best_ord[upd] = ordj[upd]

    # rebuild placement from the chosen restart per (core, iter)
    nodes_gic = np.empty((NC, ITERS, P), np.int64)
    sizes = np.zeros((NC, ITERS, 8), np.int64)
    for j in range(P):
        jj = best_ord[:, :, j]
        g = best_g[:, :, j].astype(np.int64)
        slot = sizes[ci, ii, g]
        nodes_gic[ci, ii, 16 * g + slot] = nodes_rank[ci, ii, jj]
        sizes[ci, ii, g] += 1

    # inverse: real node -> (iteration, channel)
    it_of = np.empty(N + 1, np.int32)
    ch_of = np.empty(N + 1, np.int32)
    for c in range(NC):
        flat = nodes_gic[c].reshape(-1)
        it_of[flat] = np.arange(ITERS * P) // P
        ch_of[flat] = np.arange(ITERS * P) % P

    W = posx[nodes_gic].transpose(0, 2, 1).copy()             # [NC, 128, 98]
    cntf = degx[nodes_gic].transpose(0, 2, 1).astype(np.float32)

    # per-edge placement
    e_core = dst32 // NPC
    e_it = it_of[dst32]
    e_ch = ch_of[dst32]
    e_grp = e_ch // 16
    e_r = e_ch % 16
    e_pass = src32 // CPC
    e_sidx = (src32 - e_pass * CPC).astype(np.int16)

    # group-stream length per (core, pass, group, iter) then shared width
    key = (((e_core.astype(np.int64) * PASSES + e_pass) * 8 + e_grp)
           * ITERS + e_it)
    glen = np.bincount(key, minlength=NC * PASSES * 8 * ITERS)
    glen = glen.reshape(NC, PASSES, 8, ITERS)
    # shared L per (pass, iter): max over cores and groups (no per-window
    # rounding -- masks/sidx are absolute-slot addressed; only each chunk's
    # total is padded to a multiple of 16 for the wrapped idx layout)
    L_pi = np.maximum(glen.max(axis=(0, 2)), 1).astype(np.int64)

    # chunking: pack iterations so the padded sum <= IDX_CAP
    chunks = []                                               # per pass: (it0, it1)
    for p in range(PASSES):
        ch_list = []
        it0 = 0
        while it0 < ITERS:
            tot = 0
            it1 = it0
            while it1 < ITERS and ((tot + int(L_pi[p, it1]) + 15) // 16 * 16
                                   <= IDX_CAP):
                tot += int(L_pi[p, it1])
                it1 += 1
            ch_list.append((it0, it1))
            it0 = it1
        chunks.append(ch_list)

    # column offsets per (pass, iter); chunks padded to x16
    colof = np.zeros((PASSES, ITERS), np.int64)
    chunk_cols = {}                                           # (p, it0) -> padded cols
    off = 0
    for p in range(PASSES):
        for (it0, it1) in chunks[p]:
            c0 = off
            for it in range(it0, it1):
                colof[p, it] = off
                off += int(L_pi[p, it])
            off = (off + 15) // 16 * 16
            chunk_cols[(p, it0)] = off - c0
    total_cols = off                                          # slots per group

    # edge slot position: order edges by (core, pass, grp, it, r) and number
    order = np.lexsort((e_r, e_it, e_grp, e_pass, e_core))
    ks = key[order]
    run_start = np.r_[True, ks[1:] != ks[:-1]]
    pos_in_grp = np.arange(len(order)) - np.maximum.accumulate(
        np.where(run_start, np.arange(len(order)), 0))
    # slot of each (sorted) edge within its (pass, it) stream window
    oc = e_core[order]
    op_ = e_pass[order]
    og = e_grp[order]
    oi = e_it[order]
    orr = e_r[order]
    slot = colof[op_, oi] + pos_in_grp                        # [E] global col

    # sidx [NC, 128, total_cols/16] int16, wrapped per group;
    # mask [NC, 128, total_cols] bf16
    sidx = np.zeros((NC, P, total_cols // 16), np.int16)
    mask = np.zeros((NC, P, total_cols), BF)
    # wrapped position: stream slot t -> (partition 16g + t%16, col t//16)
    sidx[oc, 16 * og + slot % 16, slot // 16] = e_sidx[order]
    mask[oc, 16 * og + orr, slot] = 1.0

    # pass tables [NC, PASSES, 128, CPC] f32 (pos chunk replicated; pad slots
    # are masked so table[0] garbage is harmless)
    tbl = np.empty((NC, PASSES, P, CPC), np.float32)
    for p in range(PASSES):
        tbl[:, p, :, :] = pos[p * CPC:(p + 1) * CPC][None, None, :]

    in_maps = []
    for c in range(NC):
        in_maps.append({
            "tbl": tbl[c].reshape(PASSES * P, CPC),
            "sidx": sidx[c],
            "mask": mask[c],
            "wtab": W[c],
            "cntf": cntf[c],
        })
    meta = dict(chunks=chunks, L_pi=L_pi, colof=colof, chunk_cols=chunk_cols,
                total_cols=int(total_cols), nodes_gic=nodes_gic)
    return in_maps, meta


def _build_program(meta):
    chunks, L_pi, total_cols = meta["chunks"], meta["L_pi"], meta["total_cols"]
    chunk_cols = meta["chunk_cols"]
    nc = bacc.Bacc()
    tbl = nc.declare_dram_parameter("tbl", [PASSES * P, CPC],
                                    mybir.dt.float32, isOutput=False)
    sidx = nc.declare_dram_parameter("sidx", [P, total_cols // 16],
                                     mybir.dt.int16, isOutput=False)
    mask = nc.declare_dram_parameter("mask", [P, total_cols],
                                     mybir.dt.bfloat16, isOutput=False)
    wtab = nc.declare_dram_parameter("wtab", [P, ITERS], mybir.dt.float32,
                                     isOutput=False)
    cntf = nc.declare_dram_parameter("cntf", [P, ITERS], mybir.dt.float32,
                                     isOutput=False)
    out = nc.declare_dram_parameter("out", [P, 2 * ITERS], mybir.dt.float32,
                                    isOutput=True)
    outv = out[:].rearrange("p (b a) -> p b a", b=2)

    with TileContext(nc) as tc:
        with (
            tc.tile_pool(name="persist", bufs=1) as pers,
            tc.tile_pool(name="tblp", bufs=2) as tblp,
            tc.tile_pool(name="idxp", bufs=4) as idxp,
            tc.tile_pool(name="maskp", bufs=3) as maskp,
            tc.tile_pool(name="vp", bufs=3) as vp,
            tc.tile_pool(name="tbp", bufs=2) as tbp,
            tc.tile_pool(name="sp", bufs=2) as sp,
        ):
            t_w = pers.tile([P, ITERS], mybir.dt.float32, tag="t_w")
            t_cnt = pers.tile([P, ITERS], mybir.dt.float32, tag="t_cnt")
            t_s = pers.tile([P, ITERS], mybir.dt.float32, tag="t_s")
            # pass-0 table first: it gates the first gather
            t_tbl0 = tblp.tile([P, CPC], mybir.dt.float32, tag="tbl")
            nc.sync.dma_start(out=t_tbl0[:], in_=tbl[0:P])
            nc.sync.dma_start(out=t_w[:], in_=wtab[:])
            nc.sync.dma_start(out=t_cnt[:], in_=cntf[:])
            nc.vector.memset(t_s[:], 0.0)

            off = 0
            for p in range(PASSES):
                if p == 0:
                    t_tbl = t_tbl0
                else:
                    t_tbl = tblp.tile([P, CPC], mybir.dt.float32, tag="tbl")
                    nc.sync.dma_start(out=t_tbl[:],
                                      in_=tbl[p * P:(p + 1) * P])
                s_cols = sp.tile([P, ITERS], mybir.dt.float32, tag="scols")
                nc.vector.memset(s_cols[:], 0.0)
                for (it0, it1) in chunks[p]:
                    Ls = [int(L_pi[p, it]) for it in range(it0, it1)]
                    cols = chunk_cols[(p, it0)]
                    si = idxp.tile([P, cols // 16], mybir.dt.int16, tag="si")
                    nc.sync.dma_start(out=si[:],
                                      in_=sidx[:, off // 16:(off + cols) // 16])
                    mk = maskp.tile([P, cols], mybir.dt.bfloat16, tag="mk")
                    nc.sync.dma_start(out=mk[:], in_=mask[:, off:off + cols])
                    v = vp.tile([P, cols], mybir.dt.float32, tag="v")
                    nc.gpsimd.ap_gather(out_ap=v[:], in_ap=t_tbl[:],
                                        idxs_ap=si[:], channels=P,
                                        num_elems=CPC, d=1, num_idxs=cols)
                    tb = tbp.tile([P, cols], mybir.dt.bfloat16, tag="tb")
                    co = 0
                    for k, it in enumerate(range(it0, it1)):
                        L = Ls[k]
                        nc.vector.tensor_scalar(
                            out=tb[:, co:co + L], in0=v[:, co:co + L],
                            scalar1=t_w[:, it:it + 1], scalar2=None,
                            op0=mybir.AluOpType.subtract)
                        nc.vector.tensor_tensor(
                            out=tb[:, co:co + L], in0=tb[:, co:co + L],
                            in1=mk[:, co:co + L], op=mybir.AluOpType.mult)
                        nc.vector.tensor_reduce(
                            out=s_cols[:, it:it + 1], in_=tb[:, co:co + L],
                            axis=mybir.AxisListType.X, op=mybir.AluOpType.add,
                            apply_absolute_value=True)
                        co += L
                    off += cols
                nc.vector.tensor_tensor(out=t_s[:], in0=t_s[:], in1=s_cols[:],
                                        op=mybir.AluOpType.add)

            nc.vector.tensor_scalar_max(out=t_cnt[:], in0=t_cnt[:],
                                        scalar1=1.0)
            nc.vector.reciprocal(out=t_cnt[:], in_=t_cnt[:])
            nc.vector.tensor_tensor(out=t_s[:], in0=t_s[:], in1=t_cnt[:],
                                    op=mybir.AluOpType.mult)
            nc.sync.dma_start(out=outv[:, 0], in_=t_w[:])
            nc.sync.dma_start(out=outv[:, 1], in_=t_s[:])

    nc.compile()
    _split_sync_waits(nc)
    return nc


def kernel(h, src, dst):
    h = np.asarray(h)
    src = np.asarray(src)
    dst = np.asarray(dst)
    in_maps, meta = _host_prep(h, src, dst)
    nc = _build_program(meta)
    res = run_bass_kernel_spmd(nc, in_maps, list(range(NC)))
    nodes_gic = meta["nodes_gic"]
    final = np.empty((N_NODES, 2), np.float32)
    for c in range(NC):
        r = res.results[c]["out"].reshape(P, 2, ITERS)
        flat_nodes = nodes_gic[c].reshape(-1)          # (it, ch) order
        vals = r.transpose(2, 0, 1).reshape(-1, 2)
        valid = flat_nodes < N_NODES                   # drop sentinel dummies
        final[flat_nodes[valid]] = vals[valid]
    return final
